# revision 10
# baseline (speedup 1.0000x reference)
"""Trainium2 Bass kernel for ComputeAlignmentError.

Math: for each (i, j) pair,
    errors[i,j] = || P_j (u_i - o_j) - T_j (v_i - q_j) + eps*1 ||
with P_j, T_j the orthonormal frame bases built from pred/true frames.
Using orthonormality, errors^2 factorizes into a K=17 inner product
    errors^2[i,j] = phi_i . psi_j
    phi = [1, ||u||^2+||v||^2, 2u, 2v, -2*(u (x) v)]          (i-side)
    psi = [c0, 1, Mq - o, M^T o - q, M]                        (j-side)
    M = P^T T,  c0 = ||o||^2 + ||q||^2 - 2 o^T M q
(the eps=1e-8 terms perturb errors by <2e-8 and are dropped).

So the device work is: small per-row feature computation (vector/scalar
engines), one K=17 fp32 matmul per output tile (tensor engine), sqrt
(scalar engine), and a 9.4 MB/core HBM output write -- the roofline.

Sharding: flat (b*n) row axis split across 8 cores; core c handles
batch c//4, rows (c%4)*768 ... +768, producing a [768, 3072] slab.
Every core needs the full frames of its batch (psi is j-side).
"""

import numpy as np

_B, _N = 2, 3072
_P = 128          # partitions
_T = _N // _P     # 24 j-subtiles per core
_S = 6            # i-subtiles per core (768 rows)
_R = _P * _S      # 768 rows per core
_K = 17           # lifted feature dim
_NCORES = 8

_cache = {}


def _build_nc():
    import concourse.mybir as mybir
    from concourse import bacc
    from concourse.masks import make_identity
    from concourse.tile import TileContext

    f32 = mybir.dt.float32
    u8 = mybir.dt.uint8
    P, T, S, K, N, R = _P, _T, _S, _K, _N, _R
    IT = 2 * T  # (instance, tile) flattened: 0..23 pred, 24..47 true

    nc = bacc.Bacc()
    pf = nc.declare_dram_parameter("pf", [N, 9], f32, isOutput=False)
    tf = nc.declare_dram_parameter("tf", [N, 9], f32, isOutput=False)
    uc = nc.declare_dram_parameter("uc", [R, 3], f32, isOutput=False)
    vc = nc.declare_dram_parameter("vc", [R, 3], f32, isOutput=False)
    mj = nc.declare_dram_parameter("mj", [N], u8, isOutput=False)
    mi = nc.declare_dram_parameter("mi", [R], u8, isOutput=False)
    out = nc.declare_dram_parameter("out", [R, N], f32, isOutput=True)

    with TileContext(nc) as tc:
        with (
            tc.tile_pool(name="const", bufs=1) as cpool,
            tc.tile_pool(name="feat", bufs=1) as fpool,
            tc.tile_pool(name="ob", bufs=4) as opool,
            tc.tile_pool(name="ps_mm", bufs=2, space="PSUM") as pmm,
            tc.tile_pool(name="ps_tr", bufs=2, space="PSUM") as ptr_,
        ):
            idn = cpool.tile([P, P], f32)
            make_identity(nc, idn[:])

            # ---- inputs -> SBUF --------------------------------------
            # frames: j index = p*T + t (contiguous 864B per partition)
            F = cpool.tile([P, 2, T, 9], f32)
            nc.sync.dma_start(
                out=F[:, 0], in_=pf[:].rearrange("(p t) f -> p t f", p=P)
            )
            nc.sync.dma_start(
                out=F[:, 1], in_=tf[:].rearrange("(p t) f -> p t f", p=P)
            )
            # coords: i_local = p*S + s
            XUV = cpool.tile([P, S, 2, 3], f32)
            nc.sync.dma_start(
                out=XUV[:, :, 0, :],
                in_=uc[:].rearrange("(p s) c -> p s c", p=P),
            )
            nc.sync.dma_start(
                out=XUV[:, :, 1, :],
                in_=vc[:].rearrange("(p s) c -> p s c", p=P),
            )
            # masks, cast u8 -> f32 during SWDGE DMA
            mjf = cpool.tile([P, T], f32)
            nc.gpsimd.dma_start(out=mjf[:], in_=mj[:].rearrange("(p t) -> p t", p=P))
            mif = cpool.tile([P, S], f32)
            nc.gpsimd.dma_start(out=mif[:], in_=mi[:].rearrange("(p s) -> p s", p=P))

            Fk = F[:].rearrange("p i t (k a) -> p i t k a", a=3)
            o_ap = Fk[:, 0, :, :, 1]        # [P, T, 3] pred origins
            q_ap = Fk[:, 1, :, :, 1]        # [P, T, 3] true origins

            # ---- frame bases (both instances stacked: IT=48) ---------
            # W[:, it, 0, :] = a - b ; W[:, it, 1, :] = c - b
            W = fpool.tile([P, IT, 2, 3], f32)
            ac = Fk[:, :, :, :, 0::2].rearrange("p i t k w -> p (i t) w k")
            bb = (
                Fk[:, :, :, :, 1]
                .rearrange("p i t k -> p (i t) k")
                .unsqueeze(2)
                .broadcast_to([P, IT, 2, 3])
            )
            nc.vector.tensor_sub(W[:], ac, bb)

            def _normalize(vecs, n2):
                # vecs: [P, IT, 2, 3] AP (possibly strided); normalizes in
                # place using reference semantics t / max(||t||, 1e-8).
                sq = fpool.tile([P, IT, 2, 3], f32, tag=f"sq{n2}")
                nc.scalar.square(sq[:], vecs)
                ss = fpool.tile([P, IT, 2], f32, tag=f"ss{n2}")
                nc.vector.tensor_add(ss[:], sq[:, :, :, 0], sq[:, :, :, 1])
                nc.vector.tensor_add(ss[:], ss[:], sq[:, :, :, 2])
                nc.scalar.sqrt(ss[:], ss[:])
                nc.vector.tensor_scalar_max(ss[:], ss[:], 1e-8)
                rcp = fpool.tile([P, IT, 2], f32, tag=f"rcp{n2}")
                nc.vector.reciprocal(rcp[:], ss[:])
                nc.vector.tensor_mul(
                    vecs, vecs, rcp[:].unsqueeze(3).broadcast_to([P, IT, 2, 3])
                )

            _normalize(W[:], 0)
            # EB holds [e1, e2] extended to 5 cols for the cross product
            EB = cpool.tile([P, IT, 2, 5], f32)
            nc.vector.tensor_add(EB[:, :, 0, 0:3], W[:, :, 0, :], W[:, :, 1, :])
            nc.vector.tensor_sub(EB[:, :, 1, 0:3], W[:, :, 1, :], W[:, :, 0, :])
            _normalize(EB[:, :, :, 0:3], 1)
            nc.gpsimd.tensor_copy(out=EB[:, :, :, 3:5], in_=EB[:, :, :, 0:2])
            # e3 = e1 x e2 (unit by construction)
            CR = fpool.tile([P, IT, 3], f32)
            nc.vector.tensor_mul(CR[:], EB[:, :, 0, 1:4], EB[:, :, 1, 2:5])
            CR2 = fpool.tile([P, IT, 3], f32)
            nc.vector.tensor_mul(CR2[:], EB[:, :, 0, 2:5], EB[:, :, 1, 1:4])
            E3 = cpool.tile([P, IT, 3], f32)
            nc.vector.tensor_sub(E3[:], CR[:], CR2[:])

            # ---- psi features [P, T, 32] (padded to 32 so transposed
            # tiles land at 32-aligned PSUM partitions) ----------------
            KP = 32
            PSI = cpool.tile([P, T, KP], f32)
            nc.gpsimd.memset(PSI[:], 0.0)
            psiq = PSI[:, :, 8:17].rearrange("p t (a b) -> p t a b", b=3)
            # M = sum_e outer(P_e, T_e)   (pred rows 0:T, true rows T:2T)
            MT1 = fpool.tile([P, T, 3, 3], f32)
            nc.vector.tensor_mul(
                MT1[:],
                EB[:, 0:T, 0, 0:3].unsqueeze(3).broadcast_to([P, T, 3, 3]),
                EB[:, T:IT, 0, 0:3].unsqueeze(2).broadcast_to([P, T, 3, 3]),
            )
            MT2 = fpool.tile([P, T, 3, 3], f32)
            nc.vector.tensor_mul(
                MT2[:],
                EB[:, 0:T, 1, 0:3].unsqueeze(3).broadcast_to([P, T, 3, 3]),
                EB[:, T:IT, 1, 0:3].unsqueeze(2).broadcast_to([P, T, 3, 3]),
            )
            nc.vector.tensor_add(MT1[:], MT1[:], MT2[:])
            MT3 = fpool.tile([P, T, 3, 3], f32)
            nc.vector.tensor_mul(
                MT3[:],
                E3[:, 0:T, :].unsqueeze(3).broadcast_to([P, T, 3, 3]),
                E3[:, T:IT, :].unsqueeze(2).broadcast_to([P, T, 3, 3]),
            )
            nc.vector.tensor_add(psiq, MT1[:], MT3[:])

            # Mq[kp] = sum_kq M[kp,kq] q[kq];  Mto[kq] = sum_kp M[kp,kq] o[kp]
            H = fpool.tile([P, T, 3, 3], f32)
            nc.vector.tensor_mul(
                H[:], psiq, q_ap.unsqueeze(2).broadcast_to([P, T, 3, 3])
            )
            Mq = fpool.tile([P, T, 3], f32)
            nc.vector.tensor_add(Mq[:], H[:, :, :, 0], H[:, :, :, 1])
            nc.vector.tensor_add(Mq[:], Mq[:], H[:, :, :, 2])
            H2 = fpool.tile([P, T, 3, 3], f32)
            nc.vector.tensor_mul(
                H2[:], psiq, o_ap.unsqueeze(3).broadcast_to([P, T, 3, 3])
            )
            Mto = fpool.tile([P, T, 3], f32)
            nc.vector.tensor_add(Mto[:], H2[:, :, 0, :], H2[:, :, 1, :])
            nc.vector.tensor_add(Mto[:], Mto[:], H2[:, :, 2, :])
            nc.vector.tensor_sub(PSI[:, :, 2:5], Mq[:], o_ap)
            nc.vector.tensor_sub(PSI[:, :, 5:8], Mto[:], q_ap)

            # c0 = ||o||^2 + ||q||^2 - 2 o.Mq
            OS = fpool.tile([P, IT, 3], f32)
            nc.scalar.square(OS[:], Fk[:, :, :, :, 1].rearrange("p i t k -> p (i t) k"))
            osum = fpool.tile([P, IT], f32)
            nc.vector.tensor_add(osum[:], OS[:, :, 0], OS[:, :, 1])
            nc.vector.tensor_add(osum[:], osum[:], OS[:, :, 2])
            OM3 = fpool.tile([P, T, 3], f32)
            nc.vector.tensor_mul(OM3[:], o_ap, Mq[:])
            oMq = fpool.tile([P, T], f32)
            nc.vector.tensor_add(oMq[:], OM3[:, :, 0], OM3[:, :, 1])
            nc.vector.tensor_add(oMq[:], oMq[:], OM3[:, :, 2])
            t1 = fpool.tile([P, T], f32)
            nc.vector.tensor_add(t1[:], osum[:, 0:T], osum[:, T:IT])
            nc.vector.scalar_tensor_tensor(
                out=PSI[:, :, 0],
                in0=oMq[:],
                scalar=-2.0,
                in1=t1[:],
                op0=mybir.AluOpType.mult,
                op1=mybir.AluOpType.add,
            )
            nc.vector.memset(PSI[:, :, 1], 1.0)
            # scale the M block by -2 now that Mq/Mto/oMq are derived from it;
            # phi's quad block stays a plain u (x) v product.
            nc.scalar.mul(PSI[:, :, 8:17], PSI[:, :, 8:17], -2.0)
            nc.vector.tensor_mul(
                PSI[:, :, 0:K],
                PSI[:, :, 0:K],
                mjf[:].unsqueeze(2).broadcast_to([P, T, K]),
            )

            # ---- phi features [P, S, 32] (same padding) --------------
            PHI = cpool.tile([P, S, KP], f32)
            nc.gpsimd.memset(PHI[:], 0.0)
            XS = fpool.tile([P, S, 2, 3], f32)
            nc.scalar.square(XS[:], XUV[:])
            t3 = fpool.tile([P, S, 3], f32)
            nc.vector.tensor_add(t3[:], XS[:, :, 0, :], XS[:, :, 1, :])
            t4 = fpool.tile([P, S], f32)
            nc.vector.tensor_add(t4[:], t3[:, :, 0], t3[:, :, 1])
            nc.vector.tensor_add(PHI[:, :, 1], t4[:], t3[:, :, 2])
            phiq = PHI[:, :, 8:17].rearrange("p s (a b) -> p s a b", b=3)
            nc.vector.tensor_mul(
                phiq,
                XUV[:, :, 0, :].unsqueeze(3).broadcast_to([P, S, 3, 3]),
                XUV[:, :, 1, :].unsqueeze(2).broadcast_to([P, S, 3, 3]),
            )
            nc.scalar.mul(PHI[:, :, 2:5], XUV[:, :, 0, :], 2.0)
            nc.scalar.mul(PHI[:, :, 5:8], XUV[:, :, 1, :], 2.0)
            nc.vector.memset(PHI[:, :, 0], 1.0)
            nc.vector.tensor_mul(
                PHI[:, :, 0:K],
                PHI[:, :, 0:K],
                mif[:].unsqueeze(2).broadcast_to([P, S, K]),
            )

            # ---- transpose features to K-major via PE ----------------
            # 4 padded tiles per transpose: [128, 128] -> PSUM [128, 128],
            # feature rows of tile m at partitions 32*m .. 32*m+16.
            phit = []  # six [17, 128] lhsT tiles
            for g in range(2):
                nt = min(4, S - 4 * g)
                ps_phi = ptr_.tile([P, P], f32, tag="pst")
                nc.tensor.transpose(
                    ps_phi[0 : KP * nt, :],
                    PHI[:, 4 * g : 4 * g + nt, :].rearrange("p s k -> p (s k)"),
                    idn[:],
                )
                for m in range(nt):
                    tl = cpool.tile([K, P], f32, tag=f"phit{4 * g + m}")
                    nc.any.tensor_copy(
                        out=tl[:], in_=ps_phi[KP * m : KP * m + K, :]
                    )
                    phit.append(tl)

            PSIT = cpool.tile([K, N], f32)
            # psi column j = p*T + t  ->  view [K, t, p]
            PSITv = PSIT[:].rearrange("k (p t) -> k t p", t=T)
            for g in range(6):
                ps_t = ptr_.tile([P, P], f32, tag="pst")
                nc.tensor.transpose(
                    ps_t[:],
                    PSI[:, 4 * g : 4 * (g + 1), :].rearrange("p t k -> p (t k)"),
                    idn[:],
                )
                for m in range(4):
                    nc.any.tensor_copy(
                        out=PSITv[:, 4 * g + m, :],
                        in_=ps_t[KP * m : KP * m + K, :],
                    )

            # ---- main loop: matmul + sqrt + store --------------------
            # out row i_local = p*S + s  ->  view [s, p, j]
            outv = out[:].rearrange("(p s) j -> s p j", s=S)
            HALF = N // 2  # 1536 columns = 3 PSUM banks
            for s in range(S):
                for h in range(2):
                    ps = pmm.tile([P, HALF], f32, tag="mm")
                    for c in range(3):
                        nc.tensor.matmul(
                            ps[:, 512 * c : 512 * (c + 1)],
                            phit[s][:],
                            PSIT[:, HALF * h + 512 * c : HALF * h + 512 * (c + 1)],
                            start=True,
                            stop=True,
                        )
                    ob = opool.tile([P, HALF], f32, tag="ob")
                    nc.scalar.sqrt(ob[:], ps[:])
                    nc.sync.dma_start(
                        out=outv[s, :, HALF * h : HALF * (h + 1)], in_=ob[:]
                    )

    nc.finalize()
    return nc


def _get_nc():
    if "nc" not in _cache:
        _cache["nc"] = _build_nc()
    return _cache["nc"]


def _make_in_maps(pred_coords, true_coords, pred_frames, true_frames, mask):
    f32 = np.float32
    pc = np.ascontiguousarray(pred_coords, dtype=f32)
    tc_ = np.ascontiguousarray(true_coords, dtype=f32)
    pfr = np.ascontiguousarray(pred_frames, dtype=f32).reshape(_B, _N, 9)
    tfr = np.ascontiguousarray(true_frames, dtype=f32).reshape(_B, _N, 9)
    m8 = np.ascontiguousarray(mask).astype(np.uint8)
    in_maps = []
    for c in range(_NCORES):
        b, r0 = c // 4, (c % 4) * _R
        in_maps.append(
            {
                "pf": pfr[b],
                "tf": tfr[b],
                "uc": pc[b, r0 : r0 + _R],
                "vc": tc_[b, r0 : r0 + _R],
                "mj": m8[b],
                "mi": m8[b, r0 : r0 + _R],
            }
        )
    return in_maps


def run(inputs, trace=False, trace_kwargs=None):
    """Run the SPMD kernel on 8 cores; returns (full_output, BassKernelResults)."""
    from concourse.bass_utils import run_bass_kernel_spmd

    nc = _get_nc()
    in_maps = _make_in_maps(**inputs)
    res = run_bass_kernel_spmd(
        nc,
        in_maps,
        list(range(_NCORES)),
        trace=trace,
        **(trace_kwargs or {}),
    )
    full = np.empty((_B, _N, _N), np.float32)
    for c in range(_NCORES):
        b, r0 = c // 4, (c % 4) * _R
        full[b, r0 : r0 + _R, :] = res.results[c]["out"]
    return full, res


def kernel(pred_coords, true_coords, pred_frames, true_frames, mask):
    full, _ = run(
        {
            "pred_coords": pred_coords,
            "true_coords": true_coords,
            "pred_frames": pred_frames,
            "true_frames": true_frames,
            "mask": mask,
        }
    )
    return full


# revision 22
# speedup vs baseline: 1.6653x; 1.6653x over previous
"""Trainium2 Bass kernel for ComputeAlignmentError.

Math: for each (i, j) pair,
    errors[i,j] = || P_j (u_i - o_j) - T_j (v_i - q_j) + eps*1 ||
with P_j, T_j the orthonormal frame bases built from pred/true frames.
Using orthonormality, errors^2 factorizes into a K=17 inner product
    errors^2[i,j] = phi_i . psi_j
    phi = [1, ||u||^2+||v||^2, 2u, 2v, -2*(u (x) v)]          (i-side)
    psi = [c0, 1, Mq - o, M^T o - q, M]                        (j-side)
    M = P^T T,  c0 = ||o||^2 + ||q||^2 - 2 o^T M q
(the eps=1e-8 terms perturb errors by <2e-8 and are dropped).

So the device work is: small per-row feature computation (vector/scalar
engines), one K=17 fp32 matmul per output tile (tensor engine), sqrt
(scalar engine), and a 9.4 MB/core HBM output write -- the roofline.

Sharding: flat (b*n) row axis split across 8 cores; core c handles
batch c//4, rows (c%4)*768 ... +768, producing a [768, 3072] slab.
Every core needs the full frames of its batch (psi is j-side).
"""

import numpy as np

_B, _N = 2, 3072
_P = 128          # partitions
_T = _N // _P     # 24 j-subtiles per core
_S = 6            # i-subtiles per core (768 rows)
_R = _P * _S      # 768 rows per core
_K = 17           # lifted feature dim
_NCORES = 8

_cache = {}
_DEBUG_NO_SQRT = False  # output raw errors^2 (skip sqrt) for precision probing


def _build_nc():
    import concourse.mybir as mybir
    from concourse import bacc
    from concourse.masks import make_identity
    from concourse.tile import TileContext

    f32 = mybir.dt.float32
    u8 = mybir.dt.uint8
    P, T, S, K, N, R = _P, _T, _S, _K, _N, _R
    IT = 2 * T  # (instance, tile) flattened: 0..23 pred, 24..47 true

    nc = bacc.Bacc()
    pf = nc.declare_dram_parameter("pf", [N, 9], f32, isOutput=False)
    tf = nc.declare_dram_parameter("tf", [N, 9], f32, isOutput=False)
    uc = nc.declare_dram_parameter("uc", [R, 3], f32, isOutput=False)
    vc = nc.declare_dram_parameter("vc", [R, 3], f32, isOutput=False)
    mj = nc.declare_dram_parameter("mj", [N], u8, isOutput=False)
    mi = nc.declare_dram_parameter("mi", [R], u8, isOutput=False)
    out = nc.declare_dram_parameter("out", [R, N], f32, isOutput=True)

    with TileContext(nc) as tc:
        with (
            tc.tile_pool(name="const", bufs=1) as cpool,
            tc.tile_pool(name="feat", bufs=1) as fpool,
            tc.tile_pool(name="ob", bufs=4) as opool,
            tc.tile_pool(name="ps_mm", bufs=2, space="PSUM") as pmm,
            tc.tile_pool(name="ps_tr", bufs=4, space="PSUM") as ptr_,
        ):
            idn = cpool.tile([P, P], f32)
            make_identity(nc, idn[:])

            # ---- inputs -> SBUF --------------------------------------
            # frames: j index = p*T + t (contiguous 864B per partition)
            F = cpool.tile([P, 2, T, 9], f32)
            nc.sync.dma_start(
                out=F[:, 0], in_=pf[:].rearrange("(p t) f -> p t f", p=P)
            )
            nc.sync.dma_start(
                out=F[:, 1], in_=tf[:].rearrange("(p t) f -> p t f", p=P)
            )
            # coords: i_local = p*S + s
            XUV = cpool.tile([P, S, 2, 3], f32)
            nc.sync.dma_start(
                out=XUV[:, :, 0, :],
                in_=uc[:].rearrange("(p s) c -> p s c", p=P),
            )
            nc.sync.dma_start(
                out=XUV[:, :, 1, :],
                in_=vc[:].rearrange("(p s) c -> p s c", p=P),
            )
            # masks, cast u8 -> f32 during SWDGE DMA
            mjf = cpool.tile([P, T], f32)
            nc.gpsimd.dma_start(out=mjf[:], in_=mj[:].rearrange("(p t) -> p t", p=P))
            mif = cpool.tile([P, S], f32)
            nc.gpsimd.dma_start(out=mif[:], in_=mi[:].rearrange("(p s) -> p s", p=P))

            Fk = F[:].rearrange("p i t (k a) -> p i t k a", a=3)
            o_ap = Fk[:, 0, :, :, 1]        # [P, T, 3] pred origins
            q_ap = Fk[:, 1, :, :, 1]        # [P, T, 3] true origins

            # ---- frame bases (both instances stacked: IT=48) ---------
            # W[:, it, 0, :] = a - b ; W[:, it, 1, :] = c - b
            W = fpool.tile([P, IT, 2, 3], f32)
            ac = Fk[:, :, :, :, 0::2].rearrange("p i t k w -> p (i t) w k")
            bb = (
                Fk[:, :, :, :, 1]
                .rearrange("p i t k -> p (i t) k")
                .unsqueeze(2)
                .broadcast_to([P, IT, 2, 3])
            )
            nc.vector.tensor_sub(W[:], ac, bb)

            def _normalize(vecs, n2):
                # vecs: [P, IT, 2, 3] AP (possibly strided); normalizes in
                # place using reference semantics t / max(||t||, 1e-8).
                sq = fpool.tile([P, IT, 2, 3], f32, tag=f"sq{n2}")
                nc.vector.tensor_mul(sq[:], vecs, vecs)
                ss = fpool.tile([P, IT, 2], f32, tag=f"ss{n2}")
                nc.vector.tensor_add(ss[:], sq[:, :, :, 0], sq[:, :, :, 1])
                nc.vector.tensor_add(ss[:], ss[:], sq[:, :, :, 2])
                nc.scalar.sqrt(ss[:], ss[:])
                nc.vector.tensor_scalar_max(ss[:], ss[:], 1e-8)
                rcp = fpool.tile([P, IT, 2], f32, tag=f"rcp{n2}")
                nc.vector.reciprocal(rcp[:], ss[:])
                nc.vector.tensor_mul(
                    vecs, vecs, rcp[:].unsqueeze(3).broadcast_to([P, IT, 2, 3])
                )

            _normalize(W[:], 0)
            # EB holds [e1, e2] extended to 5 cols for the cross product
            EB = cpool.tile([P, IT, 2, 5], f32)
            nc.vector.tensor_add(EB[:, :, 0, 0:3], W[:, :, 0, :], W[:, :, 1, :])
            nc.vector.tensor_sub(EB[:, :, 1, 0:3], W[:, :, 1, :], W[:, :, 0, :])
            _normalize(EB[:, :, :, 0:3], 1)
            nc.vector.tensor_copy(out=EB[:, :, :, 3:5], in_=EB[:, :, :, 0:2])
            # e3 = e1 x e2 (unit by construction)
            CR = fpool.tile([P, IT, 3], f32)
            nc.vector.tensor_mul(CR[:], EB[:, :, 0, 1:4], EB[:, :, 1, 2:5])
            CR2 = fpool.tile([P, IT, 3], f32)
            nc.vector.tensor_mul(CR2[:], EB[:, :, 0, 2:5], EB[:, :, 1, 1:4])
            E3 = cpool.tile([P, IT, 3], f32)
            nc.vector.tensor_sub(E3[:], CR[:], CR2[:])

            # ---- psi features [P, T, 32] (padded to 32 so transposed
            # tiles land at 32-aligned PSUM partitions; pad cols are junk
            # that the post-transpose copies never read) ----------------
            KP = 32
            PSI = cpool.tile([P, T, KP], f32)
            psiq = PSI[:, :, 8:17].rearrange("p t (a b) -> p t a b", b=3)
            # M = sum_e outer(P_e, T_e)   (pred rows 0:T, true rows T:2T)
            MT1 = fpool.tile([P, T, 3, 3], f32)
            nc.vector.tensor_mul(
                MT1[:],
                EB[:, 0:T, 0, 0:3].unsqueeze(3).broadcast_to([P, T, 3, 3]),
                EB[:, T:IT, 0, 0:3].unsqueeze(2).broadcast_to([P, T, 3, 3]),
            )
            MT2 = fpool.tile([P, T, 3, 3], f32)
            nc.vector.tensor_mul(
                MT2[:],
                EB[:, 0:T, 1, 0:3].unsqueeze(3).broadcast_to([P, T, 3, 3]),
                EB[:, T:IT, 1, 0:3].unsqueeze(2).broadcast_to([P, T, 3, 3]),
            )
            nc.vector.tensor_add(MT1[:], MT1[:], MT2[:])
            MT3 = fpool.tile([P, T, 3, 3], f32)
            nc.vector.tensor_mul(
                MT3[:],
                E3[:, 0:T, :].unsqueeze(3).broadcast_to([P, T, 3, 3]),
                E3[:, T:IT, :].unsqueeze(2).broadcast_to([P, T, 3, 3]),
            )
            nc.vector.tensor_add(psiq, MT1[:], MT3[:])

            # Mq[kp] = sum_kq M[kp,kq] q[kq];  Mto[kq] = sum_kp M[kp,kq] o[kp]
            H = fpool.tile([P, T, 3, 3], f32)
            nc.vector.tensor_mul(
                H[:], psiq, q_ap.unsqueeze(2).broadcast_to([P, T, 3, 3])
            )
            Mq = fpool.tile([P, T, 3], f32)
            nc.vector.tensor_add(Mq[:], H[:, :, :, 0], H[:, :, :, 1])
            nc.vector.tensor_add(Mq[:], Mq[:], H[:, :, :, 2])
            H2 = fpool.tile([P, T, 3, 3], f32)
            nc.vector.tensor_mul(
                H2[:], psiq, o_ap.unsqueeze(3).broadcast_to([P, T, 3, 3])
            )
            Mto = fpool.tile([P, T, 3], f32)
            nc.vector.tensor_add(Mto[:], H2[:, :, 0, :], H2[:, :, 1, :])
            nc.vector.tensor_add(Mto[:], Mto[:], H2[:, :, 2, :])
            nc.vector.tensor_sub(PSI[:, :, 2:5], Mq[:], o_ap)
            nc.vector.tensor_sub(PSI[:, :, 5:8], Mto[:], q_ap)

            # c0 = ||o||^2 + ||q||^2 - 2 o.Mq
            OS = fpool.tile([P, IT, 3], f32)
            ovw = Fk[:, :, :, :, 1].rearrange("p i t k -> p (i t) k")
            nc.vector.tensor_mul(OS[:], ovw, ovw)
            osum = fpool.tile([P, IT], f32)
            nc.vector.tensor_add(osum[:], OS[:, :, 0], OS[:, :, 1])
            nc.vector.tensor_add(osum[:], osum[:], OS[:, :, 2])
            OM3 = fpool.tile([P, T, 3], f32)
            nc.vector.tensor_mul(OM3[:], o_ap, Mq[:])
            oMq = fpool.tile([P, T], f32)
            nc.vector.tensor_add(oMq[:], OM3[:, :, 0], OM3[:, :, 1])
            nc.vector.tensor_add(oMq[:], oMq[:], OM3[:, :, 2])
            t1 = fpool.tile([P, T], f32)
            nc.vector.tensor_add(t1[:], osum[:, 0:T], osum[:, T:IT])
            nc.vector.scalar_tensor_tensor(
                out=PSI[:, :, 0],
                in0=oMq[:],
                scalar=-2.0,
                in1=t1[:],
                op0=mybir.AluOpType.mult,
                op1=mybir.AluOpType.add,
            )
            nc.vector.memset(PSI[:, :, 1], 1.0)
            # scale the M block by -2 now that Mq/Mto/oMq are derived from it;
            # phi's quad block stays a plain u (x) v product.
            nc.scalar.mul(PSI[:, :, 8:17], PSI[:, :, 8:17], -2.0)
            nc.vector.tensor_mul(
                PSI[:, :, 0:K],
                PSI[:, :, 0:K],
                mjf[:].unsqueeze(2).broadcast_to([P, T, K]),
            )

            # ---- phi features [P, S, 32] (same padding) --------------
            PHI = cpool.tile([P, S, KP], f32)
            XS = fpool.tile([P, S, 2, 3], f32)
            nc.vector.tensor_mul(XS[:], XUV[:], XUV[:])
            t3 = fpool.tile([P, S, 3], f32)
            nc.vector.tensor_add(t3[:], XS[:, :, 0, :], XS[:, :, 1, :])
            t4 = fpool.tile([P, S], f32)
            nc.vector.tensor_add(t4[:], t3[:, :, 0], t3[:, :, 1])
            nc.vector.tensor_add(PHI[:, :, 1], t4[:], t3[:, :, 2])
            phiq = PHI[:, :, 8:17].rearrange("p s (a b) -> p s a b", b=3)
            nc.vector.tensor_mul(
                phiq,
                XUV[:, :, 0, :].unsqueeze(3).broadcast_to([P, S, 3, 3]),
                XUV[:, :, 1, :].unsqueeze(2).broadcast_to([P, S, 3, 3]),
            )
            nc.scalar.mul(PHI[:, :, 2:5], XUV[:, :, 0, :], 2.0)
            nc.scalar.mul(PHI[:, :, 5:8], XUV[:, :, 1, :], 2.0)
            nc.vector.memset(PHI[:, :, 0], 1.0)
            nc.vector.tensor_mul(
                PHI[:, :, 0:K],
                PHI[:, :, 0:K],
                mif[:].unsqueeze(2).broadcast_to([P, S, K]),
            )

            # ---- transpose features to K-major via PE ----------------
            # 4 padded tiles per transpose: [128, 128] -> PSUM [128, 128],
            # feature rows of tile m at partitions 32*m .. 32*m+16.
            # The psum->sbuf copies round to fp32r, which the fp32r matmul
            # requires of its producers.
            f32r = mybir.dt.float32r
            phit = []  # six [17, 128] lhsT tiles
            for g in range(2):
                nt = min(4, S - 4 * g)
                ps_phi = ptr_.tile([P, P], f32, tag="pst")
                nc.tensor.transpose(
                    ps_phi[0 : KP * nt, :],
                    PHI[:, 4 * g : 4 * g + nt, :].rearrange("p s k -> p (s k)"),
                    idn[:],
                )
                for m in range(nt):
                    tl = cpool.tile([K, P], f32r, tag=f"phit{4 * g + m}")
                    nc.any.tensor_copy(
                        out=tl[:], in_=ps_phi[KP * m : KP * m + K, :]
                    )
                    phit.append(tl)

            PSIT = cpool.tile([K, N], f32r)
            # psi column j = p*T + t  ->  view [K, t, p]
            PSITv = PSIT[:].rearrange("k (p t) -> k t p", t=T)
            for g in range(6):
                ps_t = ptr_.tile([P, P], f32, tag="pst")
                nc.tensor.transpose(
                    ps_t[:],
                    PSI[:, 4 * g : 4 * (g + 1), :].rearrange("p t k -> p (t k)"),
                    idn[:],
                )
                for m in range(4):
                    nc.any.tensor_copy(
                        out=PSITv[:, 4 * g + m, :],
                        in_=ps_t[KP * m : KP * m + K, :],
                    )

            # ---- main loop: matmul + sqrt + store --------------------
            # fp32r: single-pass fp32 matmul (1 cycle/col at N>=256) vs
            # the 2x2-pass LOW_HIGH decomposition plain fp32 lowers to.
            # out row i_local = p*S + s  ->  view [s, p, j]
            outv = out[:].rearrange("(p s) j -> s p j", s=S)
            CH = 1024  # psum tile: 2 banks; 2 bufs + 4 transpose banks = 8
            for s in range(S):
                for h in range(N // CH):
                    ps = pmm.tile([P, CH], f32, tag="mm")
                    for c in range(CH // 512):
                        off = CH * h + 512 * c
                        nc.tensor.matmul(
                            ps[:, 512 * c : 512 * (c + 1)],
                            phit[s][:],
                            PSIT[:, off : off + 512],
                            start=True,
                            stop=True,
                        )
                    ob = opool.tile([P, CH], f32, tag="ob")
                    # fp32r rounding can push near-zero errors^2 slightly
                    # negative (measured >= -1.6e-3); clamp on DVE while
                    # moving PSUM->SBUF, then sqrt in place on ACT.
                    nc.vector.tensor_scalar_max(ob[:], ps[:], 0.0)
                    if not _DEBUG_NO_SQRT:
                        nc.scalar.sqrt(ob[:], ob[:])
                    nc.sync.dma_start(
                        out=outv[s, :, CH * h : CH * (h + 1)], in_=ob[:]
                    )

    nc.finalize()
    return nc


def _get_nc():
    if "nc" not in _cache:
        _cache["nc"] = _build_nc()
    return _cache["nc"]


def _make_in_maps(pred_coords, true_coords, pred_frames, true_frames, mask):
    f32 = np.float32
    pc = np.ascontiguousarray(pred_coords, dtype=f32)
    tc_ = np.ascontiguousarray(true_coords, dtype=f32)
    pfr = np.ascontiguousarray(pred_frames, dtype=f32).reshape(_B, _N, 9)
    tfr = np.ascontiguousarray(true_frames, dtype=f32).reshape(_B, _N, 9)
    m8 = np.ascontiguousarray(mask).astype(np.uint8)
    in_maps = []
    for c in range(_NCORES):
        b, r0 = c // 4, (c % 4) * _R
        in_maps.append(
            {
                "pf": pfr[b],
                "tf": tfr[b],
                "uc": pc[b, r0 : r0 + _R],
                "vc": tc_[b, r0 : r0 + _R],
                "mj": m8[b],
                "mi": m8[b, r0 : r0 + _R],
            }
        )
    return in_maps


def run(inputs, trace=False, trace_kwargs=None):
    """Run the SPMD kernel on 8 cores; returns (full_output, BassKernelResults)."""
    from concourse.bass_utils import run_bass_kernel_spmd

    nc = _get_nc()
    in_maps = _make_in_maps(**inputs)
    res = run_bass_kernel_spmd(
        nc,
        in_maps,
        list(range(_NCORES)),
        trace=trace,
        **(trace_kwargs or {}),
    )
    full = np.empty((_B, _N, _N), np.float32)
    for c in range(_NCORES):
        b, r0 = c // 4, (c % 4) * _R
        full[b, r0 : r0 + _R, :] = res.results[c]["out"]
    return full, res


def kernel(pred_coords, true_coords, pred_frames, true_frames, mask):
    full, _ = run(
        {
            "pred_coords": pred_coords,
            "true_coords": true_coords,
            "pred_frames": pred_frames,
            "true_frames": true_frames,
            "mask": mask,
        }
    )
    return full


# revision 24
# speedup vs baseline: 1.7668x; 1.0609x over previous
"""Trainium2 Bass kernel for ComputeAlignmentError.

Math: for each (i, j) pair,
    errors[i,j] = || P_j (u_i - o_j) - T_j (v_i - q_j) + eps*1 ||
with P_j, T_j the orthonormal frame bases built from pred/true frames.
Using orthonormality, errors^2 factorizes into a K=17 inner product
    errors^2[i,j] = phi_i . psi_j
    phi = [1, ||u||^2+||v||^2, 2u, 2v, u (x) v]                (i-side)
    psi = [c0, 1, Mq - o, M^T o - q, -2M]                      (j-side)
    M = P^T T,  c0 = ||o||^2 + ||q||^2 - 2 o^T M q
(the eps=1e-8 terms perturb errors by <2e-8 and are dropped).

Device work: per-row feature computation (vector/scalar engines), a
K=17 fp32r matmul per output tile (tensor engine), clamp + sqrt, and a
9.4 MB/core HBM output write -- the roofline.

Layout: row index i = s*128 + p, column index j = t*128 + p (partition
p fastest) -- the host interleaves frames/coords accordingly, so every
DMA is contiguous and matmul/output tiling is natural. psi features are
computed in two t-halves so the second half's feature chain overlaps
the first half's matmuls.

Sharding: flat (b*n) row axis split across 8 cores; core c handles
batch c//4, rows (c%4)*768 ... +768, producing a [768, 3072] slab.
"""

import numpy as np

_B, _N = 2, 3072
_P = 128          # partitions
_T = _N // _P     # 24 j-subtiles
_TH = _T // 2     # 12 j-subtiles per half
_S = 6            # i-subtiles per core (768 rows)
_R = _P * _S      # 768 rows per core
_K = 17           # lifted feature dim
_KP = 32          # feature dim padded for PSUM partition alignment
_NCORES = 8

_cache = {}
_DEBUG_NO_SQRT = False  # output raw errors^2 (skip sqrt) for precision probing


def _build_nc():
    import concourse.mybir as mybir
    from concourse import bacc
    from concourse.masks import make_identity
    from concourse.tile import TileContext

    f32 = mybir.dt.float32
    f32r = mybir.dt.float32r
    u8 = mybir.dt.uint8
    P, T, TH, S, K, KP, N, R = _P, _T, _TH, _S, _K, _KP, _N, _R

    nc = bacc.Bacc()
    # host-prepped layouts (pure gather/interleave, no arithmetic):
    #   fr[p, t, inst, 9]  = frames[inst][j = t*128 + p]
    #   xc[p, s, inst, 3]  = coords[inst][i = s*128 + p]
    #   mj[p, t] = mask[t*128 + p],  mi[p, s] = mask_rows[s*128 + p]
    fr = nc.declare_dram_parameter("fr", [P, T, 2, 9], f32, isOutput=False)
    xc = nc.declare_dram_parameter("xc", [P, S, 2, 3], f32, isOutput=False)
    mj = nc.declare_dram_parameter("mj", [P, T], u8, isOutput=False)
    mi = nc.declare_dram_parameter("mi", [P, S], u8, isOutput=False)
    out = nc.declare_dram_parameter("out", [R, N], f32, isOutput=True)

    with TileContext(nc) as tc:
        with (
            tc.tile_pool(name="const", bufs=1) as cpool,
            tc.tile_pool(name="feat", bufs=2) as fpool,
            tc.tile_pool(name="ob", bufs=4) as opool,
            tc.tile_pool(name="ps_mm", bufs=2, space="PSUM") as pmm,
            tc.tile_pool(name="ps_tr", bufs=2, space="PSUM") as ptr_,
        ):
            idn = cpool.tile([P, P], f32)
            make_identity(nc, idn[:])

            # ---- inputs -> SBUF (3 parallel DMA queues) --------------
            F = cpool.tile([P, T, 2, 9], f32)
            nc.sync.dma_start(out=F[:], in_=fr[:])
            XUV = cpool.tile([P, S, 2, 3], f32)
            nc.scalar.dma_start(out=XUV[:], in_=xc[:])
            mjf = cpool.tile([P, T], f32)
            nc.gpsimd.dma_start(out=mjf[:], in_=mj[:])
            mif = cpool.tile([P, S], f32)
            nc.gpsimd.dma_start(out=mif[:], in_=mi[:])

            Fk = F[:].rearrange("p t i (k a) -> p t i k a", a=3)

            PSI = cpool.tile([P, T, KP], f32)
            PSIT = cpool.tile([K, N], f32r)

            def psi_half(h):
                t0, t1 = h * TH, (h + 1) * TH
                TI = 2 * TH  # (t, inst) flattened
                Fh = Fk[:, t0:t1]                       # [P, TH, 2, 3, 3]
                o_ap = Fh[:, :, 0, :, 1]                # [P, TH, 3] pred origin
                q_ap = Fh[:, :, 1, :, 1]                # [P, TH, 3] true origin
                Fak = fr[:]  # placeholder to appease linters; unused
                del Fak

                # W[:, ti, 0, :] = a - b ; W[:, ti, 1, :] = c - b
                W = fpool.tile([P, TI, 2, 3], f32, tag="W")
                avk = F[:, t0:t1].rearrange("p t i (k a) -> p (t i) a k", a=3)
                nc.vector.tensor_sub(
                    W[:],
                    avk[:, :, 0::2, :],
                    avk[:, :, 1, :].unsqueeze(2).broadcast_to([P, TI, 2, 3]),
                )

                def _normalize(vecs, tg):
                    sq = fpool.tile([P, TI, 2, 3], f32, tag=f"sq{tg}")
                    nc.vector.tensor_mul(sq[:], vecs, vecs)
                    ss = fpool.tile([P, TI, 2], f32, tag=f"ss{tg}")
                    nc.vector.tensor_add(ss[:], sq[:, :, :, 0], sq[:, :, :, 1])
                    nc.vector.tensor_add(ss[:], ss[:], sq[:, :, :, 2])
                    nc.scalar.sqrt(ss[:], ss[:])
                    nc.vector.tensor_scalar_max(ss[:], ss[:], 1e-8)
                    rcp = fpool.tile([P, TI, 2], f32, tag=f"rcp{tg}")
                    nc.vector.reciprocal(rcp[:], ss[:])
                    nc.vector.tensor_mul(
                        vecs, vecs, rcp[:].unsqueeze(3).broadcast_to([P, TI, 2, 3])
                    )

                _normalize(W[:], "w")
                # EB holds [e1, e2] extended to 5 cols for the cross product
                EB = fpool.tile([P, TI, 2, 5], f32, tag="EB")
                nc.vector.tensor_add(EB[:, :, 0, 0:3], W[:, :, 0, :], W[:, :, 1, :])
                nc.vector.tensor_sub(EB[:, :, 1, 0:3], W[:, :, 1, :], W[:, :, 0, :])
                _normalize(EB[:, :, :, 0:3], "e")
                nc.vector.tensor_copy(out=EB[:, :, :, 3:5], in_=EB[:, :, :, 0:2])
                # e3 = e1 x e2 (unit by construction)
                CR = fpool.tile([P, TI, 3], f32, tag="CR")
                nc.vector.tensor_mul(CR[:], EB[:, :, 0, 1:4], EB[:, :, 1, 2:5])
                CR2 = fpool.tile([P, TI, 3], f32, tag="CR2")
                nc.vector.tensor_mul(CR2[:], EB[:, :, 0, 2:5], EB[:, :, 1, 1:4])
                E3 = fpool.tile([P, TI, 3], f32, tag="E3")
                nc.vector.tensor_sub(E3[:], CR[:], CR2[:])

                # per-instance views: (t i) index = t*2 + inst
                EBv = EB[:].rearrange("p (t i) e x -> p t i e x", i=2)
                E3v = E3[:].rearrange("p (t i) k -> p t i k", i=2)

                psiq = PSI[:, t0:t1, 8:17].rearrange("p t (a b) -> p t a b", b=3)
                # M = sum_e outer(P_e, T_e)
                MT1 = fpool.tile([P, TH, 3, 3], f32, tag="MT1")
                nc.vector.tensor_mul(
                    MT1[:],
                    EBv[:, :, 0, 0, 0:3].unsqueeze(3).broadcast_to([P, TH, 3, 3]),
                    EBv[:, :, 1, 0, 0:3].unsqueeze(2).broadcast_to([P, TH, 3, 3]),
                )
                MT2 = fpool.tile([P, TH, 3, 3], f32, tag="MT2")
                nc.vector.tensor_mul(
                    MT2[:],
                    EBv[:, :, 0, 1, 0:3].unsqueeze(3).broadcast_to([P, TH, 3, 3]),
                    EBv[:, :, 1, 1, 0:3].unsqueeze(2).broadcast_to([P, TH, 3, 3]),
                )
                nc.vector.tensor_add(MT1[:], MT1[:], MT2[:])
                MT3 = fpool.tile([P, TH, 3, 3], f32, tag="MT3")
                nc.vector.tensor_mul(
                    MT3[:],
                    E3v[:, :, 0, :].unsqueeze(3).broadcast_to([P, TH, 3, 3]),
                    E3v[:, :, 1, :].unsqueeze(2).broadcast_to([P, TH, 3, 3]),
                )
                nc.vector.tensor_add(psiq, MT1[:], MT3[:])

                # Mq[kp] = sum_kq M q ;  Mto[kq] = sum_kp M o
                H = fpool.tile([P, TH, 3, 3], f32, tag="H")
                nc.vector.tensor_mul(
                    H[:], psiq, q_ap.unsqueeze(2).broadcast_to([P, TH, 3, 3])
                )
                Mq = fpool.tile([P, TH, 3], f32, tag="Mq")
                nc.vector.tensor_add(Mq[:], H[:, :, :, 0], H[:, :, :, 1])
                nc.vector.tensor_add(Mq[:], Mq[:], H[:, :, :, 2])
                H2 = fpool.tile([P, TH, 3, 3], f32, tag="H2")
                nc.vector.tensor_mul(
                    H2[:], psiq, o_ap.unsqueeze(3).broadcast_to([P, TH, 3, 3])
                )
                Mto = fpool.tile([P, TH, 3], f32, tag="Mto")
                nc.vector.tensor_add(Mto[:], H2[:, :, 0, :], H2[:, :, 1, :])
                nc.vector.tensor_add(Mto[:], Mto[:], H2[:, :, 2, :])
                nc.vector.tensor_sub(PSI[:, t0:t1, 2:5], Mq[:], o_ap)
                nc.vector.tensor_sub(PSI[:, t0:t1, 5:8], Mto[:], q_ap)

                # c0 = ||o||^2 + ||q||^2 - 2 o.Mq
                OS = fpool.tile([P, TI, 3], f32, tag="OS")
                ovw = Fh[:, :, :, :, 1].rearrange("p t i k -> p (t i) k")
                nc.vector.tensor_mul(OS[:], ovw, ovw)
                osum = fpool.tile([P, TI], f32, tag="osum")
                nc.vector.tensor_add(osum[:], OS[:, :, 0], OS[:, :, 1])
                nc.vector.tensor_add(osum[:], osum[:], OS[:, :, 2])
                OM3 = fpool.tile([P, TH, 3], f32, tag="OM3")
                nc.vector.tensor_mul(OM3[:], o_ap, Mq[:])
                oMq = fpool.tile([P, TH], f32, tag="oMq")
                nc.vector.tensor_add(oMq[:], OM3[:, :, 0], OM3[:, :, 1])
                nc.vector.tensor_add(oMq[:], oMq[:], OM3[:, :, 2])
                t1s = fpool.tile([P, TH], f32, tag="t1s")
                nc.vector.tensor_add(t1s[:], osum[:, 0::2], osum[:, 1::2])
                nc.vector.scalar_tensor_tensor(
                    out=PSI[:, t0:t1, 0],
                    in0=oMq[:],
                    scalar=-2.0,
                    in1=t1s[:],
                    op0=mybir.AluOpType.mult,
                    op1=mybir.AluOpType.add,
                )
                nc.vector.memset(PSI[:, t0:t1, 1], 1.0)
                # scale M block by -2 (after Mq/Mto/oMq consumed it)
                nc.scalar.mul(PSI[:, t0:t1, 8:17], PSI[:, t0:t1, 8:17], -2.0)
                nc.vector.tensor_mul(
                    PSI[:, t0:t1, 0:K],
                    PSI[:, t0:t1, 0:K],
                    mjf[:, t0:t1].unsqueeze(2).broadcast_to([P, TH, K]),
                )

                # transpose this half's 12 tiles to K-major PSIT columns
                for g in range(3 * h, 3 * (h + 1)):
                    ps_t = ptr_.tile([P, P], f32, tag="pst")
                    nc.tensor.transpose(
                        ps_t[:],
                        PSI[:, 4 * g : 4 * (g + 1), :].rearrange(
                            "p t k -> p (t k)"
                        ),
                        idn[:],
                    )
                    for m in range(4):
                        tt = 4 * g + m
                        nc.any.tensor_copy(
                            out=PSIT[:, P * tt : P * (tt + 1)],
                            in_=ps_t[KP * m : KP * m + K, :],
                        )

            # ---- phi features [P, S, 32] -----------------------------
            def phi_side():
                PHI = cpool.tile([P, S, KP], f32)
                XS = fpool.tile([P, S, 2, 3], f32)
                nc.vector.tensor_mul(XS[:], XUV[:], XUV[:])
                t3 = fpool.tile([P, S, 3], f32)
                nc.vector.tensor_add(t3[:], XS[:, :, 0, :], XS[:, :, 1, :])
                t4 = fpool.tile([P, S], f32)
                nc.vector.tensor_add(t4[:], t3[:, :, 0], t3[:, :, 1])
                nc.vector.tensor_add(PHI[:, :, 1], t4[:], t3[:, :, 2])
                phiq = PHI[:, :, 8:17].rearrange("p s (a b) -> p s a b", b=3)
                nc.vector.tensor_mul(
                    phiq,
                    XUV[:, :, 0, :].unsqueeze(3).broadcast_to([P, S, 3, 3]),
                    XUV[:, :, 1, :].unsqueeze(2).broadcast_to([P, S, 3, 3]),
                )
                nc.scalar.mul(PHI[:, :, 2:5], XUV[:, :, 0, :], 2.0)
                nc.scalar.mul(PHI[:, :, 5:8], XUV[:, :, 1, :], 2.0)
                nc.vector.memset(PHI[:, :, 0], 1.0)
                nc.vector.tensor_mul(
                    PHI[:, :, 0:K],
                    PHI[:, :, 0:K],
                    mif[:].unsqueeze(2).broadcast_to([P, S, K]),
                )
                phit = []
                for g in range(2):
                    nt = min(4, S - 4 * g)
                    ps_phi = ptr_.tile([P, P], f32, tag="pst")
                    nc.tensor.transpose(
                        ps_phi[0 : KP * nt, :],
                        PHI[:, 4 * g : 4 * g + nt, :].rearrange("p s k -> p (s k)"),
                        idn[:],
                    )
                    for m in range(nt):
                        tl = cpool.tile([K, P], f32r, tag=f"phit{4 * g + m}")
                        nc.any.tensor_copy(
                            out=tl[:], in_=ps_phi[KP * m : KP * m + K, :]
                        )
                        phit.append(tl)
                return phit

            phit = phi_side()

            # ---- per half: features, then matmul + clamp+sqrt + store
            outv = out[:].rearrange("(s p) j -> s p j", p=P)
            CH = 1536  # psum tile: 3 banks; x2 bufs + 2 transpose banks = 8
            for h in range(2):
                psi_half(h)
                for s in range(S):
                    ps = pmm.tile([P, CH], f32, tag="mm")
                    for c in range(CH // 512):
                        off = CH * h + 512 * c
                        nc.tensor.matmul(
                            ps[:, 512 * c : 512 * (c + 1)],
                            phit[s][:],
                            PSIT[:, off : off + 512],
                            start=True,
                            stop=True,
                        )
                    ob = opool.tile([P, CH], f32, tag="ob")
                    # fp32r rounding can push near-zero errors^2 slightly
                    # negative (measured >= -1.6e-3); clamp on DVE while
                    # moving PSUM->SBUF, then sqrt in place on ACT.
                    nc.vector.tensor_scalar_max(ob[:], ps[:], 0.0)
                    if not _DEBUG_NO_SQRT:
                        nc.scalar.sqrt(ob[:], ob[:])
                    nc.sync.dma_start(
                        out=outv[s, :, CH * h : CH * (h + 1)], in_=ob[:]
                    )

    nc.finalize()
    return nc


def _get_nc():
    if "nc" not in _cache:
        _cache["nc"] = _build_nc()
    return _cache["nc"]


def _make_in_maps(pred_coords, true_coords, pred_frames, true_frames, mask):
    f32 = np.float32
    P, T, S, R, N, B = _P, _T, _S, _R, _N, _B
    pc = np.asarray(pred_coords, dtype=f32)
    tcc = np.asarray(true_coords, dtype=f32)
    pfr = np.asarray(pred_frames, dtype=f32).reshape(B, N, 9)
    tfr = np.asarray(true_frames, dtype=f32).reshape(B, N, 9)
    m8 = np.asarray(mask).astype(np.uint8)

    in_maps = []
    for c in range(_NCORES):
        b, r0 = c // 4, (c % 4) * R
        # fr[p, t, inst, 9]: frames[j = t*128 + p]
        fr = np.empty((P, T, 2, 9), f32)
        fr[:, :, 0, :] = pfr[b].reshape(T, P, 9).transpose(1, 0, 2)
        fr[:, :, 1, :] = tfr[b].reshape(T, P, 9).transpose(1, 0, 2)
        # xc[p, s, inst, 3]: coords[i = r0 + s*128 + p]
        xcs = np.empty((P, S, 2, 3), f32)
        xcs[:, :, 0, :] = pc[b, r0 : r0 + R].reshape(S, P, 3).transpose(1, 0, 2)
        xcs[:, :, 1, :] = tcc[b, r0 : r0 + R].reshape(S, P, 3).transpose(1, 0, 2)
        in_maps.append(
            {
                "fr": np.ascontiguousarray(fr),
                "xc": np.ascontiguousarray(xcs),
                "mj": np.ascontiguousarray(m8[b].reshape(T, P).T),
                "mi": np.ascontiguousarray(m8[b, r0 : r0 + R].reshape(S, P).T),
            }
        )
    return in_maps


def run(inputs, trace=False, trace_kwargs=None):
    """Run the SPMD kernel on 8 cores; returns (full_output, BassKernelResults)."""
    from concourse.bass_utils import run_bass_kernel_spmd

    nc = _get_nc()
    in_maps = _make_in_maps(**inputs)
    res = run_bass_kernel_spmd(
        nc,
        in_maps,
        list(range(_NCORES)),
        trace=trace,
        **(trace_kwargs or {}),
    )
    full = np.empty((_B, _N, _N), np.float32)
    for c in range(_NCORES):
        b, r0 = c // 4, (c % 4) * _R
        full[b, r0 : r0 + _R, :] = res.results[c]["out"]
    return full, res


def kernel(pred_coords, true_coords, pred_frames, true_frames, mask):
    full, _ = run(
        {
            "pred_coords": pred_coords,
            "true_coords": true_coords,
            "pred_frames": pred_frames,
            "true_frames": true_frames,
            "mask": mask,
        }
    )
    return full
